# revision 9
# baseline (speedup 1.0000x reference)
"""HMM log-domain forward algorithm on 8 Trainium2 NeuronCores.

Strategy (pure data parallel, 32 sequences per core):
  - Scaled linear-domain forward algorithm:
        alpha_t = diag(E[:, x_t]) @ A @ alpha_{t-1}
    One TensorE matmul per step with FIXED stationary W = [A^T | ones]
    (the ones column yields per-sequence state-sums for free since the
    softmax columns of A preserve sums), then one VectorE multiply with
    the gathered emission tile while copying PSUM -> SBUF.
  - Emissions gathered host-side into [64, 32] bf16 tiles per step and
    streamed (dominant, fully-overlapped memory traffic).
  - Sequences shorter than T_MAX padded with emission prob 1.0: the
    final state-sum then equals the sum at t = T[b]-1 exactly.
  - Emission table pre-scaled by exp(-mean(logE)) => zero-drift random
    walk; per-sequence rescale (divide by running state-sum, log added
    back at the end) every 64 steps keeps values in range.

Uses bacc.Bacc (not bass.Bass): TRN2 instructions hold at most ONE sync
wait; Bacc.compile() runs move_matmul_waits_to_ldweights +
generate_event_semaphores to split multi-wait instructions legally.
"""

import math
import os

import numpy as np
import ml_dtypes

N_STATES = 64
N_OBS = 10000
BATCH = 256
T_MAX = 2048
N_CORES = 8
BPC = BATCH // N_CORES  # 32 sequences per core
BLK = 64                # time steps per emission DMA block
NBLK = T_MAX // BLK     # 32
RESCALE = 64            # rescale period (steps)
N_EVT = T_MAX // RESCALE  # 32 slots: 31 mid-run rescales + final sum

_BF16 = ml_dtypes.bfloat16

_nc_cache = {}


def _build_nc():
    """Build the per-core Bass program (same program on all 8 cores)."""
    import concourse.bass as bass
    import concourse.mybir as mybir
    import concourse.tile as tile
    from concourse import bacc

    nc = bacc.Bacc("TRN2", target_bir_lowering=False)

    egath = nc.dram_tensor(
        "egath", [NBLK, N_STATES, BLK * BPC], mybir.dt.bfloat16,
        kind="ExternalInput",
    )
    w_in = nc.dram_tensor(
        "w", [N_STATES, N_STATES + 1], mybir.dt.bfloat16, kind="ExternalInput"
    )
    out = nc.dram_tensor("out", [1, BPC], mybir.dt.float32, kind="ExternalOutput")

    f32 = mybir.dt.float32
    bf16 = mybir.dt.bfloat16

    with tile.TileContext(nc) as tc:
        with (
            tc.tile_pool(name="const", bufs=1) as cpool,
            tc.tile_pool(name="eblk", bufs=3) as epool,
            tc.tile_pool(name="state", bufs=1) as spool,
            tc.tile_pool(name="evt", bufs=2) as vpool,
            tc.tile_pool(name="ps", bufs=2, space=bass.MemorySpace.PSUM) as ppool,
            tc.tile_pool(name="psb", bufs=1, space=bass.MemorySpace.PSUM) as bpool,
        ):
            wt = cpool.tile([N_STATES, N_STATES + 1], bf16)
            nc.sync.dma_start(wt[:], w_in[:])
            ones_row = cpool.tile([1, N_STATES], bf16)
            nc.vector.memset(ones_row[:], 1.0)

            # running per-sequence scaled alpha  [state, seq]
            alpha = spool.tile([N_STATES, BPC], bf16)
            # stored rescale divisors: [1, seq, event]
            s_buf = spool.tile([1, BPC, N_EVT], f32)

            for blk in range(NBLK):
                et = epool.tile([N_STATES, BLK * BPC], bf16, tag="eblk")
                nc.sync.dma_start(et[:], egath[blk, :, :])
                if blk == 0:
                    # alpha_0 = pi * E[:, x_0] (pi folded host-side into col 0)
                    nc.vector.tensor_copy(alpha[:], et[:, 0:BPC])
                for ti in range(BLK):
                    t = blk * BLK + ti
                    if t == 0:
                        continue
                    ps = ppool.tile([N_STATES + 1, BPC], f32, tag="ps")
                    nc.tensor.matmul(ps[:], wt[:], alpha[:], start=True, stop=True)
                    # alpha_t = (A @ alpha_{t-1}) * E_t
                    nc.vector.tensor_mul(
                        alpha[:], ps[0:N_STATES, :], et[:, ti * BPC:(ti + 1) * BPC]
                    )
                    if t % RESCALE == 0:
                        evt = t // RESCALE - 1  # 0..30
                        # s = sum_k alpha_{t-1}[k, b]  (psum row 64)
                        nc.vector.tensor_copy(
                            s_buf[0:1, :, evt], ps[N_STATES:N_STATES + 1, :]
                        )
                        r32 = vpool.tile([1, BPC], f32, tag="r32")
                        nc.vector.reciprocal(r32[:], ps[N_STATES:N_STATES + 1, :])
                        r16 = vpool.tile([1, BPC], bf16, tag="r16")
                        nc.vector.tensor_copy(r16[:], r32[:])
                        # broadcast 1/s across the 64 state partitions via PE
                        rbc = bpool.tile([N_STATES, BPC], f32, tag="rbc")
                        nc.tensor.matmul(
                            rbc[:], ones_row[:], r16[:], start=True, stop=True
                        )
                        # fold 1/s into the next step's emission tile
                        nc.vector.tensor_mul(
                            et[:, (ti + 1) * BPC:(ti + 2) * BPC],
                            rbc[:],
                            et[:, (ti + 1) * BPC:(ti + 2) * BPC],
                        )

            # final state-sum
            ps = ppool.tile([N_STATES + 1, BPC], f32, tag="ps")
            nc.tensor.matmul(ps[:], wt[:], alpha[:], start=True, stop=True)
            nc.vector.tensor_copy(
                s_buf[0:1, :, N_EVT - 1], ps[N_STATES:N_STATES + 1, :]
            )

            # logp_dev[b] = sum_e log(s_buf[b, e])
            logs = spool.tile([1, BPC, N_EVT], f32)
            nc.scalar.activation(
                logs[:], s_buf[:], mybir.ActivationFunctionType.Ln
            )
            lp = spool.tile([1, BPC], f32)
            nc.vector.tensor_reduce(
                lp[:], logs[0:1, :, :], axis=mybir.AxisListType.X,
                op=mybir.AluOpType.add,
            )
            nc.sync.dma_start(out[:], lp[:])

    nc.compile()
    return nc


def _get_nc():
    if "nc" not in _nc_cache:
        _nc_cache["nc"] = _build_nc()
    return _nc_cache["nc"]


def kernel(x, T, pi, unnormalized_transition_matrix, unnormalized_emission_matrix):
    from concourse.bass_utils import run_bass_kernel_spmd

    x = np.asarray(x).astype(np.int64)
    T = np.asarray(T).astype(np.int64)
    pi = np.asarray(pi, dtype=np.float64)
    Au = np.asarray(unnormalized_transition_matrix, dtype=np.float64)
    Eu = np.asarray(unnormalized_emission_matrix, dtype=np.float64)

    # --- host-side parameter prep ---
    Am = Au - Au.max(axis=0, keepdims=True)
    A = np.exp(Am)
    A /= A.sum(axis=0, keepdims=True)
    W = np.concatenate([A.T, np.ones((N_STATES, 1))], axis=1).astype(_BF16)

    Em = Eu - Eu.max(axis=1, keepdims=True)
    logZ = np.log(np.exp(Em).sum(axis=1, keepdims=True))
    logE = Em - logZ                      # [64, N_OBS] log softmax rows
    m = float(logE.mean())
    Epre = np.exp(logE - m).astype(np.float32)            # [64, N_OBS]
    Epre = np.concatenate(
        [Epre, np.ones((N_STATES, 1), np.float32)], axis=1
    )  # padding symbol N_OBS -> emission prob 1.0

    pi_lin = np.exp(pi - pi.max())
    pi_lin = (pi_lin / pi_lin.sum() * N_STATES).astype(np.float32)  # [64]

    tgrid = np.arange(T_MAX)[None, :]
    xp = np.where(tgrid < T[:, None], x, N_OBS)

    in_maps = []
    for c in range(N_CORES):
        xc = xp[c * BPC:(c + 1) * BPC]            # [32, 2048]
        G = Epre[:, xc]                           # [64, 32, 2048] (n, b, t)
        G[:, :, 0] *= pi_lin[:, None]
        G = G.reshape(N_STATES, BPC, NBLK, BLK)   # [n, b, blk, ti]
        G = np.ascontiguousarray(G.transpose(2, 0, 3, 1))  # [blk, n, ti, b]
        egath_c = G.reshape(NBLK, N_STATES, BLK * BPC).astype(_BF16)
        in_maps.append({"egath": egath_c, "w": W})

    nc = _get_nc()
    trace = bool(int(os.environ.get("HMM_KERNEL_TRACE", "0")))
    try:
        res = run_bass_kernel_spmd(
            nc, in_maps, core_ids=list(range(N_CORES)), trace=trace,
        )
    except ModuleNotFoundError:
        # axon NTFF profile hook unavailable in this container; rerun untraced
        os.environ["BASS_NEVER_TRACE"] = "1"
        res = run_bass_kernel_spmd(
            nc, in_maps, core_ids=list(range(N_CORES)), trace=False,
        )
    _nc_cache["last_results"] = res

    dev = np.concatenate([r["out"][0] for r in res.results])  # [256]
    logp = dev.astype(np.float64) - math.log(N_STATES) + m * T.astype(np.float64)
    return logp[:, None].astype(np.float32)
